# revision 64
# baseline (speedup 1.0000x reference)
"""Causal self-attention (B=2, T=2048, C=1024, 16 heads x 64) on 8 TRN2 cores.

Sharding: tensor-parallel over heads (2 heads/core). Each core computes its
heads' QKV projection, causal attention, and a partial output projection
(contraction over its 128 attn columns); the host sums the 8 partials.

Final design (chunk-pipelined, engine-balanced; 250.6us -> 180.4us):
  - x DMA'd per 512-token chunk (c-halves split so the first projection
    matmul starts early); chunk n+1's V/Q/K projection matmuls are
    emitted as PE "filler" quanta inside attention qc=n, so the exp-gated
    PE gaps get real work, input DMA fully overlaps compute (~27us head
    -> ~12us), and the HAM clock-throttle rarely re-arms (PE stays at
    2.4GHz for >100us stretches).
  - Scores st tiles [P,2(head),512] f32, one per k-block; exp is trimmed
    to the live causal range on diagonal blocks and split per head there
    so head 0's mask+PV overlap head 1's exp.
  - Causal mask multiplies run on the otherwise-idle GpSimd engine.
  - 1/l broadcast: reciprocal (fp32) is cast to bf16 before the sel2
    broadcast matmul so it avoids the 2-pass fp32 PE mode (~16us saved).
  - Output projection: two N=512 matmuls per token block, evacuated as
    bf16 (halves out DMA); host sums the 8 partials in f32. attnT/l
    drains split across Scalar+Vector; the final batch tail spreads its
    copies over both engines and its DMAs over two descriptors, with
    keepalive matmuls holding PE activity through the drain chain.
  - First batch's tail (norm + out-projection of its last 512 tokens)
    is carried into the second batch's first attention block as mid-loop
    extras (cross-batch overlap).
  - V transposed per chunk (PE transpose); both heads' slices land in
    one strided Vector copy per token block.
"""

import os
from collections import deque

import numpy as np
import ml_dtypes

B = 2
T = 2048
C = 1024
N_HEADS = 16
D = 64
NCORES = 8
P = 128
BT = B * T
SCALE = D ** -0.5
NCH = 4          # 512-token chunks per batch
CW = T // NCH    # chunk width (= qc width)

_bf16 = ml_dtypes.bfloat16
_f8 = ml_dtypes.float8_e4m3

_COMPILED = None
LAST_RESULTS = None  # stashed BassKernelResults for test harness introspection


def _build():
    import concourse.bass as bass
    import concourse.mybir as mybir
    import concourse.tile as tile
    from concourse import bacc

    f32 = mybir.dt.float32
    bf16 = mybir.dt.bfloat16
    f8 = mybir.dt.float8e4
    DR = mybir.MatmulPerfMode.DoubleRow

    nc = bacc.Bacc("TRN2", target_bir_lowering=False, debug=False,
                   num_devices=NCORES)

    xT_d = nc.dram_tensor("xT", [P, 8, BT], bf16, kind="ExternalInput")
    wqkvT_d = nc.dram_tensor("wqkvT", [P, 3, 8, 128], bf16,
                         kind="ExternalInput")
    woutT_d = nc.dram_tensor("woutT", [P, C], bf16, kind="ExternalInput")
    maskT_d = nc.dram_tensor("maskT", [P, P], bf16, kind="ExternalInput")
    sel2_d = nc.dram_tensor("sel2", [65, P], bf16, kind="ExternalInput")
    ident_d = nc.dram_tensor("ident", [P, P], bf16, kind="ExternalInput")
    out_d = nc.dram_tensor("out", [BT, C], bf16, kind="ExternalOutput")

    Exp = mybir.ActivationFunctionType.Exp

    with tile.TileContext(nc) as tc:
        with (
            tc.tile_pool(name="const", bufs=1) as const_pool,
            tc.tile_pool(name="xn", bufs=4) as xn_pool,
            tc.tile_pool(name="seq", bufs=2) as seq_pool,
            tc.tile_pool(name="vtn", bufs=3) as vtn_pool,
            tc.tile_pool(name="pt", bufs=6) as pt_pool,
            tc.tile_pool(name="rlb", bufs=2) as rlb_pool,
            tc.tile_pool(name="osb", bufs=4) as osb_pool,
            tc.tile_pool(name="st", bufs=2, space="PSUM") as st_pool,
            tc.tile_pool(name="pv", bufs=2, space="PSUM") as pv_pool,
            tc.tile_pool(name="ps", bufs=2, space="PSUM") as ps_pool,
        ):
            wqkvT = const_pool.tile([P, 3, 8, 128], bf16, tag="wqkvT")
            ident = const_pool.tile([P, P], bf16, tag="ident")
            maskT = const_pool.tile([P, P], bf16, tag="maskT")
            sel2 = const_pool.tile([65, P], bf16, tag="sel2")
            woutT = const_pool.tile([P, C], bf16, tag="woutT")
            # v-slice of the weights + ident first so chunk 0's V projection
            # can start as early as possible; the rest follows behind.
            nc.sync.dma_start(wqkvT[:, 2], wqkvT_d[:, 2])
            nc.sync.dma_start(ident, ident_d[:])

            fillers = deque()  # of (key, fn)

            def drain(k):
                n = 0
                while fillers and n < k:
                    fillers.popleft()[1]()
                    n += 1

            def force(key):
                # emit (in order) until no entry with this key remains
                while any(e[0] == key for e in fillers):
                    fillers.popleft()[1]()

            def flush():
                drain(len(fillers))

            def make_state(b):
                S = {}
                S["qT"] = seq_pool.tile([P, T], bf16, tag="qT",
                                        name=f"qT{b}")
                S["kT"] = seq_pool.tile([P, T], bf16, tag="kT",
                                        name=f"kT{b}")
                S["attnT"] = seq_pool.tile([P, T], bf16, tag="attnT",
                                           name=f"attnT{b}")
                S["vb"] = seq_pool.tile([P, 16, 2, 65], bf16, tag="vb",
                                        name=f"vb{b}")
                nc.vector.memset(S["vb"][:, :, :, 64], 1.0)
                S["l2"] = seq_pool.tile([65, T], f32, tag="l2",
                                        name=f"l2{b}")
                S["rl2"] = seq_pool.tile([65, T], f32, tag="rl2",
                                         name=f"rl2{b}")
                # rows 1-63 feed zero sel2 rows; 1.0 keeps 1/x finite there
                nc.vector.memset(S["l2"], 1.0)
                return S

            def make_chunk_quanta(b, n, S):
                """Queue chunk n's DMA now; return PE quanta closures."""
                xn = xn_pool.tile([P, 8, CW], bf16, tag="xn",
                                  name=f"x{b}_{n}")
                tsl = slice(b * T + n * CW, b * T + (n + 1) * CW)
                nc.sync.dma_start(xn[:, 0:4, :], xT_d[:, 0:4, tsl])
                nc.sync.dma_start(xn[:, 4:8, :], xT_d[:, 4:8, tsl])
                nsl = slice(n * CW, (n + 1) * CW)
                loc = {}

                def proj_pair(key, pi, c0):
                    def f():
                        if c0 == 0:
                            loc[key] = ps_pool.tile(
                                [P, CW], f32, tag="ps",
                                name=f"ps{key}{b}{n}")
                        ps = loc[key]
                        for c in range(c0, c0 + 2):
                            nc.tensor.matmul(ps, wqkvT[:, pi, c, :],
                                             xn[:, c, :],
                                             start=(c == 0), stop=(c == 7))
                    return f

                def v_fin():
                    vtn = vtn_pool.tile([P, CW], bf16, tag="vtn",
                                        name=f"vtn{b}{n}")
                    loc["vtn"] = vtn
                    nc.vector.tensor_copy(vtn, loc["v"])

                def t_one(j):
                    def f():
                        tp = ps_pool.tile([P, 2, 64], bf16, tag="ps",
                                          name=f"tp{b}{n}{j}")
                        nc.tensor.transpose(
                            tp[:, :, :],
                            loc["vtn"][:, j * 128:(j + 1) * 128], ident)
                        nc.vector.tensor_copy(
                            S["vb"][:, 4 * n + j, :, 0:64], tp[:, :, :])
                    return f

                def q_fin():
                    nc.scalar.copy(S["qT"][:, nsl], loc["q"])

                def k_fin():
                    nc.scalar.copy(S["kT"][:, nsl], loc["k"])

                def chain(f, g):
                    return lambda: (f(), g())

                VP, QP, KP = 2, 0, 1  # VP unused (fp8 path)
                # v first so its transposes overlap the q/k accumulations
                kk = (b, n)
                qs = [(kk, proj_pair("v", VP, c)) for c in (0, 2, 4)]
                qs.append((kk, chain(proj_pair("v", VP, 6), v_fin)))
                qs.extend((kk, t_one(j)) for j in range(4))
                qs.extend((kk, proj_pair("q", QP, c)) for c in (0, 2, 4))
                qs.append((kk, chain(proj_pair("q", QP, 6), q_fin)))
                qs.extend((kk, proj_pair("k", KP, c)) for c in (0, 2, 4))
                qs.append((kk, chain(proj_pair("k", KP, 6), k_fin)))
                return qs

            def emit_oproj(b, tb, S, tail=False):
                osb = osb_pool.tile([P, C], bf16, tag="osb",
                                    name=f"osb{b}{tb}")
                rows = slice(b * T + tb * 128, b * T + (tb + 1) * 128)
                for half in range(2):
                    csl = slice(half * 512, (half + 1) * 512)
                    opx = ps_pool.tile([P, 512], f32, tag="ps",
                                       name=f"op{b}{tb}{half}")
                    nc.tensor.matmul(opx,
                                     S["attnT"][:, tb * 128:(tb + 1) * 128],
                                     woutT[:, csl], start=True, stop=True)
                    if tail and half == 0:
                        # spread tail evacuation over Scalar+Vector and two
                        # DMA queues to shorten the serial endgame
                        nc.scalar.copy(osb[:, csl], opx)
                        nc.sync.dma_start(out_d[rows, csl], osb[:, csl])
                    else:
                        nc.vector.tensor_copy(osb[:, csl], opx)
                        if tail:
                            nc.sync.dma_start(out_d[rows, csl], osb[:, csl])
                if not tail:
                    nc.sync.dma_start(out_d[rows, :], osb)

            def emit_norm(b, qc, S):
                # normalize attnT[:, qc] by 1/l via PE broadcast + DVE mul
                qsl = slice(qc * CW, (qc + 1) * CW)
                rb = ps_pool.tile([P, CW], f32, tag="ps",
                                  name=f"rb{b}{qc}")
                nc.tensor.matmul(rb, sel2[:, :], S["rlb"],
                                 start=True, stop=True)
                nc.vector.tensor_mul(S["attnT"][:, qsl], S["attnT"][:, qsl],
                                     rb)

            def emit_qc(b, qc, S, extra_ops=None):
                qsl = slice(qc * CW, (qc + 1) * CW)
                nk = 4 * qc + 4
                qT, kT = S["qT"], S["kT"]
                force((b, qc))
                pv = [pv_pool.tile([P, CW], f32, tag="pv",
                                   name=f"pv{b}{qc}{h}") for h in range(2)]
                opq = deque(range(4 * (qc - 1), 4 * qc)) if qc > 0 else \
                    deque()
                extra_ops = extra_ops if extra_ops is not None else deque()
                for kb in range(nk):
                    off = max(0, (kb - 4 * qc) * 128)
                    st = st_pool.tile([P, 2, CW], f32, tag="st",
                                      name=f"st{b}{qc}{kb}")
                    pt = pt_pool.tile([P, 2, CW], bf16, tag="pt",
                                      name=f"pt{b}{qc}{kb}")
                    for h in range(2):
                        hs = h * 64
                        nc.tensor.matmul(
                            st[:, h, off:CW],
                            kT[hs:hs + 64, kb * 128:(kb + 1) * 128],
                            qT[hs:hs + 64, qc * CW + off:(qc + 1) * CW],
                            start=True, stop=True)
                    if kb >= 4 * qc:
                        # joint trimmed exp (Scalar is the hot engine);
                        # masks follow on GpSimd
                        nc.scalar.activation(pt[:, :, off:CW],
                                             st[:, :, off:CW],
                                             Exp, scale=SCALE)
                        for h in range(2):
                            nc.gpsimd.tensor_mul(
                                pt[:, h, off:off + 128],
                                pt[:, h, off:off + 128], maskT)
                    else:
                        nc.scalar.activation(pt[:, :, off:CW],
                                             st[:, :, off:CW],
                                             Exp, scale=SCALE)
                    if kb == 0 and qc > 0:
                        emit_norm(b, qc - 1, S)
                    drain(1)
                    if extra_ops and kb > 0:
                        extra_ops.popleft()()
                    for h in range(2):
                        nc.tensor.matmul(
                            pv[h][:65, off:CW], S["vb"][:, kb, h, :],
                            pt[:, h, off:CW],
                            start=(kb == 0), stop=(kb == nk - 1),
                            skip_group_check=True)
                    if opq and kb % 2 == 1:
                        emit_oproj(b, opq.popleft(), S)
                # drain: split the two l2 copies across Vector+Scalar so
                # the reciprocal chain starts one copy earlier. attnT
                # evacuation: on qc<3 it isn't latency-critical (its
                # out-projection runs one qc later), so it goes to Vector
                # AFTER recip/cast, keeping Scalar free for the exp
                # stream; on qc==3 it gates the batch tail, so it runs on
                # Scalar in parallel with recip/cast instead.
                nc.vector.tensor_copy(S["l2"][0:1, qsl], pv[0][64:65, :])
                nc.scalar.copy(S["l2"][64:65, qsl], pv[1][64:65, :])
                nc.scalar.copy(S["attnT"][0:64, qsl], pv[0][0:64, :])
                nc.scalar.copy(S["attnT"][64:128, qsl], pv[1][0:64, :])
                nc.vector.reciprocal_approx_fast(S["rl2"][:, qsl],
                                                 S["l2"][:, qsl])
                # bf16 copy of 1/l so the broadcast matmul avoids fp32 mode
                rlb = rlb_pool.tile([65, CW], bf16, tag="rlb",
                                    name=f"rlb{b}{qc}")
                nc.vector.tensor_copy(rlb, S["rl2"][:, qsl])
                S["rlb"] = rlb
                while opq:
                    emit_oproj(b, opq.popleft(), S)
                while extra_ops:
                    extra_ops.popleft()()
                if len(fillers) < 2:
                    # filler-starved qc end: keepalive matmuls hold PE
                    # activity through the drain chain (HAM stays warm)
                    for i in range(3):
                        ka = ps_pool.tile([P, CW], f32, tag="ps",
                                          name=f"ka{b}{qc}{i}")
                        nc.tensor.matmul(ka, sel2[:, :], qT[0:65, qsl],
                                         start=True, stop=True)
                flush()

            states = [make_state(b) for b in range(B)]

            chunk0 = make_chunk_quanta(0, 0, states[0])
            nc.sync.dma_start(wqkvT[:, 0:2], wqkvT_d[:, 0:2])
            nc.sync.dma_start(maskT, maskT_d[:])
            nc.sync.dma_start(sel2, sel2_d[:])
            nc.sync.dma_start(woutT, woutT_d[:])
            fillers.extend(chunk0)
            fillers.extend(make_chunk_quanta(0, 1, states[0]))

            # (batch, chunk) to queue at the start of each qc step
            nxt = deque([(0, 2), (0, 3), (1, 0), (1, 1),
                         (1, 2), (1, 3), None, None])
            carry = deque()
            for b in range(B):
                for qc in range(4):
                    nx = nxt.popleft()
                    if nx is not None:
                        fillers.extend(
                            make_chunk_quanta(nx[0], nx[1], states[nx[0]]))
                    emit_qc(b, qc, states[b], carry if qc == 0 else None)
                # batch tail: normalize qc=3 + its out-projection. For the
                # first batch these become mid-qc extras inside the next
                # batch's qc0 (cross-batch overlap); the last batch emits
                # directly with the copy/DMA work spread over two engines.
                Sb = states[b]
                if b < B - 1:
                    carry = deque(
                        [lambda b=b, Sb=Sb: emit_norm(b, 3, Sb)] +
                        [lambda b=b, tb=tb, Sb=Sb: emit_oproj(b, tb, Sb)
                         for tb in range(12, 16)])
                else:
                    # keepalive matmuls: hold PE activity (and the HAM
                    # clock at 2.4GHz) through the final drain chain so
                    # the tail norm/out-projection doesn't run throttled
                    for i in range(10):
                        ka = ps_pool.tile([P, CW], f32, tag="ps",
                                          name=f"ka{i}")
                        nc.tensor.matmul(ka, sel2[:, :],
                                         Sb["qT"][0:65, 0:CW],
                                         start=True, stop=True)
                    emit_norm(b, 3, Sb)
                    for tb in range(12, 16):
                        emit_oproj(b, tb, Sb, tail=True)

    nc.compile()
    return nc


def _get_compiled():
    global _COMPILED
    if _COMPILED is None:
        _COMPILED = _build()
    return _COMPILED


def make_core_inputs(x, w_qkv, w_out):
    """Host-side shard prep: returns list of per-core input dicts."""
    xf = np.asarray(x, dtype=np.float32).reshape(BT, C)
    xT = np.ascontiguousarray(
        xf.T.reshape(8, P, BT).transpose(1, 0, 2)).astype(_bf16)

    maskT = np.zeros((P, P), dtype=_bf16)
    kk, qq = np.meshgrid(np.arange(P), np.arange(P), indexing="ij")
    maskT[kk <= qq] = 1.0

    sel2 = np.zeros((65, P), dtype=_bf16)
    sel2[0, 0:64] = 1.0
    sel2[64, 64:128] = 1.0

    ident = np.eye(P, dtype=_bf16)

    w_qkv = np.asarray(w_qkv, dtype=np.float32)
    w_out = np.asarray(w_out, dtype=np.float32)

    ins = []
    for core in range(NCORES):
        r0 = 2 * core * D
        wsel = np.stack([
            w_qkv[r0:r0 + 128],
            w_qkv[C + r0:C + r0 + 128],
            w_qkv[2 * C + r0:2 * C + r0 + 128],
        ], axis=0)  # [3, 128, 1024]
        # -> [ci=128, proj=3, c-chunk=8, 128] (proj-major, contiguous DMA)
        wqkvT = np.ascontiguousarray(
            wsel.transpose(2, 0, 1).reshape(8, P, 3, P)
            .transpose(1, 2, 0, 3)).astype(_bf16)
        woutT = np.ascontiguousarray(
            w_out[:, core * P:(core + 1) * P].T).astype(_bf16)
        ins.append({
            "xT": xT,
            "wqkvT": wqkvT,
            "woutT": woutT,
            "maskT": maskT,
            "sel2": sel2,
            "ident": ident,
        })
    return ins


def kernel(x, w_qkv, w_out):
    global LAST_RESULTS
    from concourse.bass_utils import run_bass_kernel_spmd

    nc = _get_compiled()
    ins = make_core_inputs(x, w_qkv, w_out)
    trace = bool(os.environ.get("KERNEL_TRACE"))
    res = run_bass_kernel_spmd(nc, ins, core_ids=list(range(NCORES)),
                               trace=trace)
    LAST_RESULTS = res
    out = np.zeros((BT, C), dtype=np.float32)
    for r in res.results:
        out += np.asarray(r["out"], dtype=np.float32)
    return out.reshape(B, T, C)
